# revision 7
# baseline (speedup 1.0000x reference)
"""Trainium2 Bass kernel for nn_CPLinear (CP-decomposed QKV projection with RoPE).

Restructured for continuous PE occupancy (baseline 194us -> ~167us):
  - Every dma_start costs the issuing engine ~0.7us, so the design is built
    around a small per-tile issue budget spread over ACT/SP HWDGE + gpsimd.
  - Loads split across both HWDGE queues (together they saturate the
    ~358GB/s core HBM bw): x in SH-halves (tiles 0-3 gated by only 2.1MB),
    w by (column-chunk x ktile-quarter) in consumption order; tiles 0-1
    interleaved chunk-wise to cover the DMA lead-in.
  - per-512-col PSUM chunks with immediate eviction; 2x2-bank psq.
  - bk merged into the Bq rope pass as a 13th r-slice; ak/av evicted as one.
  - q written as contiguous [128,1024] half-dumps, host permutes.
  - Bounce: ONE identity write [128,1856] (A' rides along); readback as
    per-token gathers emitted one tile ahead of the consume so transfers
    hide under s1 compute; tile 7's bounce/readback pulled early to cut
    the drain; consume lags s1 by 2 tiles.
"""

import sys

for _p in ("/opt/trn_rl_repo",):
    if _p not in sys.path:
        sys.path.insert(0, _p)

import numpy as np
import ml_dtypes

BF16 = ml_dtypes.bfloat16

SH = 1024          # tokens per core
H = 2048           # hidden
KT = H // 128      # 16 k-tiles
NT = SH // 128     # 8 token tiles per core
NOUT = 2016        # fused projection output width
NH, HD, RQ = 16, 128, 12

# column chunks of the fused projection: c0 = A'/ak/av/bk/bv, c1-3 = Bq
CHUNKS = [(0, 480), (480, 992), (992, 1504), (1504, 2016)]

_CACHE = {}


def make_nc():
    import concourse.bacc as bacc
    from concourse import mybir

    dt = mybir.dt

    nc = bacc.Bacc(
        "TRN2",
        target_bir_lowering=False,
        debug=False,
        enable_asserts=False,
        num_devices=8,
    )

    x_d = nc.dram_tensor("x", (H, SH), dt.bfloat16, kind="ExternalInput")
    w_d = nc.dram_tensor("w", (KT, 128, NOUT), dt.bfloat16, kind="ExternalInput")
    # cos/sin pre-laid-out host-side as [128, NT*64] (SBUF image)
    cos_d = nc.dram_tensor("cosr", (128, NT * 64), dt.bfloat16,
                           kind="ExternalInput")
    sin_d = nc.dram_tensor("sinr", (128, NT * 64), dt.bfloat16,
                           kind="ExternalInput")
    q_d = nc.dram_tensor("q", (SH, NH * HD), dt.bfloat16, kind="ExternalOutput")
    k_d = nc.dram_tensor("k", (SH, NH * HD), dt.bfloat16, kind="ExternalOutput")
    v_d = nc.dram_tensor("v", (SH, NH * HD), dt.bfloat16, kind="ExternalOutput")
    return nc, (x_d, w_d, cos_d, sin_d, q_d, k_d, v_d)


def build_body(nc, tc, tensors):
    from contextlib import ExitStack

    from concourse import mybir

    dt = mybir.dt
    x_d, w_d, cos_d, sin_d, q_d, k_d, v_d = tensors

    with ExitStack() as ctx:
        P = ctx.enter_context
        const_pool = P(tc.tile_pool(name="const", bufs=1))
        w_sb = const_pool.tile([128, KT * NOUT], dt.bfloat16, tag="w_sb")
        cos_sb = const_pool.tile([128, NT * 64], dt.bfloat16, tag="cos_sb")
        sin_sb = const_pool.tile([128, NT * 64], dt.bfloat16, tag="sin_sb")
        xT = const_pool.tile([128, KT * SH], dt.bfloat16, tag="xT")
        lhs_bufs = [
            const_pool.tile([128, 2048], dt.bfloat16, tag=f"lhs{i}",
                            name=f"lhs{i}")
            for i in range(3)
        ]
        bdr_bufs = [
            const_pool.tile([128, 2048], dt.bfloat16, tag=f"bdr{i}",
                            name=f"bdr{i}")
            for i in range(3)
        ]

        # ---- constant loads, in exact PE consumption order ----
        # cos/sin first on the (otherwise idle at t=0) SP queue.
        nc.sync.dma_start(out=cos_sb[:], in_=cos_d[:])
        nc.sync.dma_start(out=sin_sb[:], in_=sin_d[:])

        w_v = w_sb[:].rearrange("p (k n) -> p k n", k=KT)
        wd_v = w_d[:].rearrange("k p n -> p k n")
        x_v = xT[:].rearrange("p (k t) -> p k t", k=KT)
        xd_v = x_d[:].rearrange("(k p) t -> p k t", p=128)

        def load_w(eng, c, k_lo, k_hi):
            lo, hi = CHUNKS[c]
            eng.dma_start(
                out=w_v[:, k_lo:k_hi, lo:hi],
                in_=wd_v[:, k_lo:k_hi, lo:hi],
            )

        def load_x(eng, k_lo, k_hi, t_lo, t_hi):
            eng.dma_start(
                out=x_v[:, k_lo:k_hi, t_lo:t_hi],
                in_=xd_v[:, k_lo:k_hi, t_lo:t_hi],
            )

        # Loads split across both HWDGE queues (the pair saturates the
        # ~358GB/s core HBM bandwidth). x comes in SH-halves so tiles 0-3
        # are gated by only 2.1MB of x; w chunks follow in consumption
        # order, quartered so partial ktiles unlock early matmuls.
        load_x(nc.scalar, 0, 2, 0, 512)
        load_w(nc.scalar, 0, 0, 4)
        load_x(nc.scalar, 2, 4, 0, 512)
        load_w(nc.scalar, 0, 4, 8)
        load_x(nc.scalar, 4, 6, 0, 512)
        load_x(nc.scalar, 6, 8, 0, 512)
        load_w(nc.scalar, 1, 0, 4)
        load_w(nc.scalar, 1, 4, 8)
        load_w(nc.scalar, 2, 0, 8)
        load_w(nc.scalar, 3, 0, 8)
        load_x(nc.sync, 8, 10, 0, 512)
        load_w(nc.sync, 0, 8, 12)
        load_x(nc.sync, 10, 12, 0, 512)
        load_w(nc.sync, 0, 12, 16)
        load_x(nc.sync, 12, 14, 0, 512)
        load_x(nc.sync, 14, 16, 0, 512)
        load_w(nc.sync, 1, 8, 12)
        load_w(nc.sync, 1, 12, 16)
        load_w(nc.sync, 2, 8, 16)
        load_w(nc.sync, 3, 8, 16)
        for kk in range(8, 16, 2):
            load_x(nc.sync, kk, kk + 2, 512, 1024)

        # lhs needs zero off-diagonal blocks; bdr is fully overwritten.
        for tl in lhs_bufs:
            nc.gpsimd.memset(tl[:], 0.0)

        ps_pool = P(tc.tile_pool(name="ps", bufs=4, space="PSUM"))
        psq_pool = P(tc.tile_pool(name="psq", bufs=2, space="PSUM"))
        bqk_pool = P(tc.tile_pool(name="bqk", bufs=3))
        bqr_pool = P(tc.tile_pool(name="bqr", bufs=3))
        tmp_pool = P(tc.tile_pool(name="tmp", bufs=3))
        small_pool = P(tc.tile_pool(name="small", bufs=4))
        out_pool = P(tc.tile_pool(name="outs", bufs=3))
        dram_pool = P(tc.tile_pool(name="scr", bufs=3, space="DRAM"))

        state = {}

        def s1(it, c):
            """projection matmuls for tile `it`, column chunk `c` + evict."""
            t0 = it * 128
            lo, hi = CHUNKS[c]
            n = hi - lo
            ps = ps_pool.tile([128, 512], dt.float32, tag="ps",
                              name=f"ps_{it}_{c}")
            for kk in range(KT):
                nc.tensor.matmul(
                    ps[:, 0:n],
                    xT[:, kk * SH + t0: kk * SH + t0 + 128],
                    w_sb[:, kk * NOUT + lo: kk * NOUT + hi],
                    start=(kk == 0),
                    stop=(kk == KT - 1),
                )

            if c == 0:
                st = state[it]
                bqk, bqr = st["bqk"], st["bqr"]
                akv_sb = small_pool.tile([128, 32], dt.bfloat16, tag="akv_sb")
                bv_sb = small_pool.tile([128, 128], dt.bfloat16, tag="bv_sb")
                nc.scalar.copy(bqr[:, 1664:1856], ps[:, 0:192])    # A'
                nc.scalar.copy(akv_sb[:], ps[:, 192:224])          # ak|av
                nc.scalar.copy(bqk[:, 1536:1664], ps[:, 224:352])  # bk
                nc.scalar.copy(bv_sb[:], ps[:, 352:480])
                st["akv"], st["bv"] = akv_sb, bv_sb
            else:
                bqk = state[it]["bqk"]
                nc.scalar.copy(bqk[:, (c - 1) * 512: c * 512], ps[:])

        def rope(it, lo, hi):
            """RoPE cols [lo,hi) of bqk -> bqr (nr r-slices of 128)."""
            st = state[it]
            bqk, bqr = st["bqk"], st["bqr"]
            nr = (hi - lo) // 128
            t_a = tmp_pool.tile([128, 320], dt.bfloat16, tag="t_a")
            t_b = tmp_pool.tile([128, 320], dt.bfloat16, tag="t_b")
            bqv = bqk[:, lo:hi].rearrange(
                "p (r two d) -> p r two d", two=2, d=64
            )
            bqrv = bqr[:, lo:hi].rearrange(
                "p (r two d) -> p r two d", two=2, d=64
            )
            cos_t = (cos_sb[:, it * 64:(it + 1) * 64]
                     .unsqueeze(1).broadcast_to([128, nr, 64]))
            sin_t = (sin_sb[:, it * 64:(it + 1) * 64]
                     .unsqueeze(1).broadcast_to([128, nr, 64]))
            tav = t_a[:, 0:nr * 64].rearrange("p (r d) -> p r d", r=nr)
            tbv = t_b[:, 0:nr * 64].rearrange("p (r d) -> p r d", r=nr)
            p_lo = bqv[:, :, 0]
            p_hi = bqv[:, :, 1]
            nc.vector.tensor_mul(tav, p_lo, cos_t)
            nc.vector.tensor_mul(tbv, p_hi, sin_t)
            nc.vector.tensor_sub(bqrv[:, :, 0], tav, tbv)
            nc.vector.tensor_mul(tav, p_hi, cos_t)
            nc.vector.tensor_mul(tbv, p_lo, sin_t)
            nc.vector.tensor_add(bqrv[:, :, 1], tav, tbv)

        def start_tile(it):
            state[it] = {
                "bqk": bqk_pool.tile([128, 1664], dt.bfloat16, tag="bqk",
                                     name=f"bqk{it}"),
                "bqr": bqr_pool.tile([128, 1856], dt.bfloat16, tag="bqr",
                                     name=f"bqr{it}"),
            }

        def bounce(it, both_sync=False):
            """bounce write: identity DMAs of bqr (A' rides along unless it
            was already written by early_lhs)."""
            st = state[it]
            bqr = st["bqr"]
            if "scr" in st:
                scr = st["scr"]
                eng = nc.sync if both_sync else nc.scalar
                eng.dma_start(out=scr[:, 0:768], in_=bqr[:, 0:768])
                nc.sync.dma_start(out=scr[:, 768:1536], in_=bqr[:, 768:1536])
                return
            scr = dram_pool.tile([128, 1856], dt.bfloat16, tag="scr_b",
                                 name=f"scr{it}")
            eng = nc.sync if both_sync else nc.scalar
            eng.dma_start(out=scr[:, 0:928], in_=bqr[:, 0:928])
            nc.sync.dma_start(out=scr[:, 928:1856], in_=bqr[:, 928:1856])
            st["scr"] = scr

        def early_lhs(it):
            """A'-region bounce + lhs scatter right after the c0 eviction,
            so the tail consume isn't gated on this serial chain."""
            st = state[it]
            scr = dram_pool.tile([128, 1856], dt.bfloat16, tag="scr_b",
                                 name=f"scr{it}")
            st["scr"] = scr
            nc.sync.dma_start(out=scr[:, 1664:1856],
                              in_=st["bqr"][:, 1664:1856])
            lhs = lhs_bufs[it % 3]
            sa_v = scr[:, 1664:1856].rearrange(
                "(g t) (r h) -> t r g h", t=8, r=RQ
            )
            l_v = lhs[0:96, :].rearrange("(t r) (g c) -> t r g c", t=8, g=16)
            for t in range(8):
                nc.gpsimd.dma_start(
                    out=l_v[t][:, :, t * 16:(t + 1) * 16], in_=sa_v[t]
                )
            st["lhs"] = lhs

        def fin(it):
            """k/v products + outputs + scatter bounce for tile `it`."""
            t0 = it * 128
            st = state[it]
            bqr = st["bqr"]
            lhs = lhs_bufs[it % 3]
            bdr = bdr_bufs[it % 3]

            # ---- k, v ----
            ksb = out_pool.tile([128, 2048], dt.bfloat16, tag="ksb")
            vsb = out_pool.tile([128, 2048], dt.bfloat16, tag="vsb")
            akv = st["akv"]
            nc.vector.tensor_mul(
                ksb[:].rearrange("p (h d) -> p h d", h=NH),
                bqr[:, 1536:1664].unsqueeze(1).broadcast_to([128, NH, 128]),
                akv[:, 0:16].unsqueeze(2).broadcast_to([128, NH, 128]),
            )
            nc.vector.tensor_mul(
                vsb[:].rearrange("p (h d) -> p h d", h=NH),
                st["bv"].unsqueeze(1).broadcast_to([128, NH, 128]),
                akv[:, 16:32].unsqueeze(2).broadcast_to([128, NH, 128]),
            )
            nc.scalar.dma_start(out=k_d[t0:t0 + 128, :], in_=ksb[:])
            nc.sync.dma_start(out=v_d[t0:t0 + 128, :], in_=vsb[:])

            # ---- bounce write (may have been emitted early, see bounce) ----
            if "scr" not in st:
                bounce(it)
            scr = st["scr"]

            # ---- lhs scatter (small, gpsimd SWDGE) ----
            if "lhs" not in st:
                sa_v = scr[:, 1664:1856].rearrange(
                    "(g t) (r h) -> t r g h", t=8, r=RQ
                )
                l_v = lhs[0:96, :].rearrange(
                    "(t r) (g c) -> t r g c", t=8, g=16
                )
                for t in range(8):
                    nc.gpsimd.dma_start(
                        out=l_v[t][:, :, t * 16:(t + 1) * 16], in_=sa_v[t]
                    )
                st["lhs"] = lhs
            st["bdr"] = bdr
            st["scr"] = scr

        def consume(it):
            """q contraction + output DMA for tile `it`."""
            t0 = it * 128
            st = state.pop(it)
            lhs, bdr = st["lhs"], st["bdr"]

            qsb = out_pool.tile([128, 2048], dt.bfloat16, tag="qsb")
            for half in range(2):
                qp = psq_pool.tile([128, 1024], dt.float32, tag="qp",
                                   name=f"qp{it}_{half}")
                for j in range(8):
                    g = half * 8 + j
                    nc.tensor.matmul(
                        qp[:, j * 128:(j + 1) * 128],
                        lhs[0:96, g * 128:(g + 1) * 128],
                        bdr[0:96, g * 128:(g + 1) * 128],
                        start=True,
                        stop=True,
                    )
                nc.scalar.copy(
                    qsb[:, half * 1024:(half + 1) * 1024], qp[:]
                )
                # contiguous dump per half, issued as soon as its eviction
                # lands; host permutes (t,h),(g,d) -> token-major
                eng = nc.sync if half == 0 else nc.scalar
                eng.dma_start(
                    out=q_d[t0:t0 + 128, half * 1024:(half + 1) * 1024],
                    in_=qsb[:, half * 1024:(half + 1) * 1024],
                )

        def read_back(it):
            """bdr readback for tile `it`, split across both HWDGE queues.
            Emitted one tile ahead of consume() so the transfers overlap
            remaining s1 compute; by emission time the scr-write sems are
            satisfied, so neither issue stream blocks."""
            st = state[it]
            scr, bdr = st["scr"], bdr_bufs[it % 3]
            sb_v = scr[:, 0:1536].rearrange(
                "(g t) (r d) -> t r g d", t=8, r=RQ
            )
            bd_v = bdr[0:96, :].rearrange("(t r) (g d) -> t r g d", t=8, g=16)
            for t in range(8):
                eng = nc.scalar if t % 2 == 0 else nc.sync
                eng.dma_start(out=bd_v[t], in_=sb_v[t])

        # ---- schedule ----
        def stage(it, c):
            if c == 0:
                start_tile(it)
            s1(it, c)
            if c == 1:
                rope(it, 0, 512)
            elif c == 2:
                rope(it, 512, 1024)
            elif c == 3:
                rope(it, 1024, 1664)   # r 8..11 + bk as 13th slice

        for it, c in [(0, 0), (1, 0), (0, 1), (1, 1), (0, 2), (1, 2), (0, 3)]:
            stage(it, c)
            if c == 0:
                early_lhs(it)
            if (it, c) == (1, 1):
                # deferred ACT x-h1 loads (tiles 4-7), after early evictions
                for kk in range(0, 8, 2):
                    load_x(nc.scalar, kk, kk + 2, 512, 1024)
        fin(0)
        stage(1, 3)
        fin(1)
        read_back(0)
        for it in range(2, 8):
            for c in range(4):
                stage(it, c)
                if c == 0:
                    early_lhs(it)
            if it == 7:
                # last tile: bounce write as early as possible so the final
                # read_back/consume chain isn't gated on queue backlog
                bounce(7, both_sync=True)
            consume(it - 2)
            read_back(it - 1)
            if it == 7:
                read_back(7)
            fin(it)
        consume(6)
        consume(7)


def build_program():
    import concourse.tile as tile

    nc, tensors = make_nc()
    with tile.TileContext(nc) as tc:
        build_body(nc, tc, tensors)
    nc.compile()
    return nc


def _get_program():
    if "nc" not in _CACHE:
        _CACHE["nc"] = build_program()
    return _CACHE["nc"]


def make_in_maps(x, W_A_q, W_B_q, W_A_k, W_B_k, W_A_v, W_B_v):
    """Shard + preprocess full inputs into per-core input maps."""
    x = np.asarray(x)
    B, S, Hh = x.shape
    x2 = np.ascontiguousarray(x.reshape(B * S, Hh))

    # fold the 1/RQ scale and the (h,r)->(r,h) column reorder into W_A_q
    WAq = np.asarray(W_A_q).reshape(Hh, NH, RQ).transpose(0, 2, 1).reshape(
        Hh, NH * RQ
    ) / np.float32(RQ)
    Wall = np.concatenate(
        [
            WAq,
            np.asarray(W_A_k),
            np.asarray(W_A_v),
            np.asarray(W_B_k),
            np.asarray(W_B_v),
            np.asarray(W_B_q),
        ],
        axis=1,
    )
    assert Wall.shape == (Hh, NOUT)
    Wt = np.ascontiguousarray(Wall.reshape(KT, 128, NOUT)).astype(BF16)

    inv = 1.0 / (10000.0 ** (np.arange(0, HD, 2, dtype=np.float32) / HD))
    ang = np.arange(S, dtype=np.float32)[:, None] * inv[None, :]
    cos_rep = np.ascontiguousarray(np.cos(ang)).astype(BF16)
    sin_rep = np.ascontiguousarray(np.sin(ang)).astype(BF16)

    def sbuf_img(tab, pos):
        # (SH, 64) -> SBUF image [128, NT*64]: partition = token%128
        t = tab[pos].reshape(NT, 128, 64).transpose(1, 0, 2)
        return np.ascontiguousarray(t.reshape(128, NT * 64))

    in_maps = []
    for i in range(8):
        tok0 = i * SH
        pos = np.arange(tok0, tok0 + SH) % S
        in_maps.append(
            {
                "x": np.ascontiguousarray(x2[tok0: tok0 + SH].T).astype(BF16),
                "w": Wt,
                "cosr": sbuf_img(cos_rep, pos),
                "sinr": sbuf_img(sin_rep, pos),
            }
        )
    return in_maps, (B, S)


def assemble_outputs(results, B, S):
    # q rows are [tile](t,h) x cols (g,d); token = tile*128 + g*8 + t
    qs = []
    for i in range(8):
        a = results[i]["q"].astype(np.float32)
        a = a.reshape(NT, 8, 16, 16, 128).transpose(0, 3, 1, 2, 4)
        qs.append(a.reshape(SH, NH, HD))
    q = np.concatenate(qs, axis=0).reshape(B, S, NH, HD)
    k = np.concatenate(
        [results[i]["k"].astype(np.float32) for i in range(8)], axis=0
    ).reshape(B, S, NH, HD)
    v = np.concatenate(
        [results[i]["v"].astype(np.float32) for i in range(8)], axis=0
    ).reshape(B, S, NH, HD)
    return q, k, v


def kernel(x, W_A_q, W_B_q, W_A_k, W_B_k, W_A_v, W_B_v):
    from concourse.bass_utils import run_bass_kernel_spmd

    nc = _get_program()
    in_maps, (B, S) = make_in_maps(x, W_A_q, W_B_q, W_A_k, W_B_k, W_A_v, W_B_v)
    res = run_bass_kernel_spmd(nc, in_maps, list(range(8))).results
    return assemble_outputs(res, B, S)
